# revision 12
# baseline (speedup 1.0000x reference)
"""CoAttention ImageDNS kernel for Trainium2 (8 NeuronCores, Bass/Tile).

Math: the reference computes two additive-attention blocks. In both, the
softmax'd score is  score[b, q, k] = f(q-side)[b, q] + g(k-side)[b, k] + c,
and softmax over k is invariant to the q-dependent (and constant) terms, so
the attention weights are independent of the query index:

  visual_att[b, s, :]  = softmax_r( wB . tanh(W_i1 @ img[b, r]) )
  textual_att[b, i, :] = softmax_j( wD . tanh(W_d2 @ dns[b, j]) )

Hence both outputs are per-batch rank-1 broadcasts:

  att_img_features[b, s, :] = visual_att[b]  @ img[b]   (same for all s)
  att_dns_features[b, i, :] = textual_att[b] @ dns[b]   (same for all i)

W_d1/b_d1/w_att1[:H]/b_att1/W_i2/b_i2/w_att2[:H]/b_att2 cancel entirely.

Sharding: pure data-parallel over batch, 4 batches per core, no collectives.
The device computes the per-batch [H] attention outputs; the host broadcasts
them over the (identical) S query rows, so the kernel writes only B*H values
instead of B*S*H.

Layout: projections run transposed (weights stationary, activations moving):
proj^T[o, r] accumulates per 128-wide o-chunk over the h-chunks, so the
score reduction over o is 8 tiny PE matmuls against the w-column and the
scores land in ROW layout on partition 0.  From there: exp (+softmax sum via
accum) on Scalar, partition-broadcast of the weight row on GpSimd, stage-2
weighted row sums on Vector (stt accum over the same transposed activation
tiles the projections use - the natural-layout activations are never
loaded).  The PE stream is pure projection + score matmuls; each group's
softmax/stage-2 tail is emitted one group later so it hides under the next
group's projections.  The img side (2-batch groups) runs first so the first
matmul only waits on ~1MB of DMA; the last dns batch is split into two
256-row sub-groups so the final exposed tail is one half-batch of vector
work.  HBM in is ~10MB/core.
"""

import sys
import numpy as np
import ml_dtypes

_BF16 = ml_dtypes.bfloat16

for _p in ("/opt/trn_rl_repo", "/root/.axon_site/_ro/trn_rl_repo"):
    if _p not in sys.path:
        sys.path.append(_p)

B, S, R, H = 32, 512, 196, 1024
NCORES = 8
BLOC = B // NCORES          # batches per core
HC = H // 128               # contraction chunks of 128
OC = H // 128               # projection output chunks of 128
NI = BLOC * R               # img rows, all batches packed (784)
ND = BLOC * S               # dns rows, all batches packed (2048)
GI = 2 * R                  # img group rows (2 batches)
HS = S // 2                 # dns sub-group rows for the final batch (256)

_CACHE = {}


def build_nc():
    from concourse import bacc, mybir
    from concourse import tile

    f32, f16 = mybir.dt.float32, mybir.dt.bfloat16
    Act = mybir.ActivationFunctionType
    Alu = mybir.AluOpType

    nc = bacc.Bacc("TRN2", target_bir_lowering=False, debug=False)

    xt_img = nc.dram_tensor("xt_img", [HC, 128, NI], f16, kind="ExternalInput")
    xt_dns = nc.dram_tensor("xt_dns", [HC, 128, ND], f16, kind="ExternalInput")
    # oc-major weight layout: [oc, hc, 128(h), 128(o)] so one 0.25MB DMA
    # delivers everything one proj accumulation group needs
    wt_i1 = nc.dram_tensor("wt_i1", [OC, HC, 128, 128], f16, kind="ExternalInput")
    wt_d2 = nc.dram_tensor("wt_d2", [OC, HC, 128, 128], f16, kind="ExternalInput")
    wcol_b = nc.dram_tensor("wcol_b", [128, OC], f16, kind="ExternalInput")
    wcol_d = nc.dram_tensor("wcol_d", [128, OC], f16, kind="ExternalInput")
    out_dns = nc.dram_tensor("out_dns", [BLOC, H], f32, kind="ExternalOutput")
    out_img = nc.dram_tensor("out_img", [BLOC, H], f32, kind="ExternalOutput")

    with tile.TileContext(nc) as tc:
        with (
            tc.tile_pool(name="const", bufs=1) as cpool,
            tc.tile_pool(name="work", bufs=3) as wpool,
            tc.tile_pool(name="small", bufs=8) as spool,
            tc.tile_pool(name="ppd", bufs=3, space="PSUM") as ppd,
            tc.tile_pool(name="ppi", bufs=3, space="PSUM") as ppi,
            tc.tile_pool(name="psr", bufs=2, space="PSUM") as psr,
        ):
            xt_i = cpool.tile([128, HC * NI], f16, name="xt_img_sb")
            xt_d = cpool.tile([128, HC * ND], f16, name="xt_dns_sb")
            wt_sb = {"img": cpool.tile([128, OC * HC * 128], f16, name="wt_i1_sb"),
                     "dns": cpool.tile([128, OC * HC * 128], f16, name="wt_d2_sb")}
            wc_sb = {"img": cpool.tile([128, OC], f16, name="wcol_b_sb"),
                     "dns": cpool.tile([128, OC], f16, name="wcol_d_sb")}

            wt_dram = {"img": wt_i1, "dns": wt_d2}
            wc_dram = {"img": wcol_b, "dns": wcol_d}
            xt_dram = {"img": xt_img, "dns": xt_dns}
            out_d = {"img": out_img, "dns": out_dns}
            n_rows = {"img": R, "dns": S}

            def load_wt_oc(side, oc):
                w = wt_sb[side]
                nc.sync.dma_start(
                    out=w[:, oc * HC * 128:(oc + 1) * HC * 128]
                    .rearrange("p (hc m) -> p hc m", hc=HC),
                    in_=wt_dram[side][oc].rearrange("hc p m -> p hc m"))

            def load_xt(side, c0, c1, h0=0, h1=HC):
                nc.sync.dma_start(
                    out=xt3[side][:, h0:h1, c0:c1],
                    in_=xt_dram[side][h0:h1, :, c0:c1]
                    .rearrange("hc p m -> p hc m"))

            wt3 = {s: wt_sb[s].rearrange("p (oc hc m) -> p oc hc m", oc=OC, hc=HC)
                   for s in ("img", "dns")}
            xt3 = {"img": xt_i.rearrange("p (hc m) -> p hc m", hc=HC),
                   "dns": xt_d.rearrange("p (hc m) -> p hc m", hc=HC)}

            # groups: (side, row0, row1, tail segments (batch, off, nr, mode))
            # mode: "full" = standalone batch, "a"/"b" = halves of dns b3
            groups = [
                ("img", 0 * GI, 1 * GI, [(0, 0, R, "full"), (1, R, R, "full")]),
                ("img", 1 * GI, 2 * GI, [(2, 0, R, "full"), (3, R, R, "full")]),
                ("dns", 0 * S, 1 * S, [(0, 0, S, "full")]),
                ("dns", 1 * S, 2 * S, [(1, 0, S, "full")]),
                ("dns", 2 * S, 3 * S, [(2, 0, S, "full")]),
                ("dns", 3 * S, 3 * S + HS, [(3, 0, HS, "a")]),
                ("dns", 3 * S + HS, 4 * S, [(3, 0, HS, "b")]),
            ]

            def emit_loads(gi):
                side = groups[gi][0]
                if gi == 0:
                    load_wt_oc("img", 0)
                    # halves so the first proj matmuls start sooner
                    load_xt("img", 0, GI, 0, HC // 2)
                    load_xt("img", 0, GI, HC // 2, HC)
                    nc.sync.dma_start(out=wc_sb["img"][:, :],
                                      in_=wc_dram["img"][:, :])
                    for oc in range(1, OC):
                        load_wt_oc("img", oc)
                elif gi == 1:
                    load_xt("img", GI, NI)
                    nc.sync.dma_start(out=wc_sb["dns"][:, :],
                                      in_=wc_dram["dns"][:, :])
                elif gi == 2:
                    load_xt("dns", 0, S)
                    for oc in range(OC):
                        load_wt_oc("dns", oc)
                elif gi in (3, 4):
                    g0, g1 = groups[gi][1], groups[gi][2]
                    load_xt("dns", g0, g1)
                elif gi == 5:
                    load_xt("dns", 3 * S, 4 * S)

            # dedicated state for the split final dns batch
            b3 = {}

            def emit_group(gi, prev_score7, prev_tail):
                side, g0, g1, _ = groups[gi]
                ng = g1 - g0
                pp = ppd if side == "dns" else ppi
                npp = S if side == "dns" else GI
                srow = psr.tile([1, S], f32, name=f"srow_{gi}", tag="srow")
                ths = []

                def score_mm(oc):
                    nc.tensor.matmul(
                        srow[0:1, 0:ng], lhsT=wc_sb[side][:, oc:oc + 1],
                        rhs=ths[oc][:, 0:ng],
                        start=(oc == 0), stop=(oc == OC - 1))

                emit_loads(gi)
                for oc in range(OC):
                    ps = pp.tile([128, npp], f32, name=f"proj_{gi}_{oc}",
                                 tag=f"pp_{side}")
                    for hc in range(HC):
                        nc.tensor.matmul(
                            ps[:, 0:ng],
                            lhsT=wt3[side][:, oc, hc, :],
                            rhs=xt3[side][:, hc, g0:g1],
                            start=(hc == 0), stop=(hc == HC - 1))
                    th = wpool.tile([128, npp], f16, name=f"th_{gi}_{oc}",
                                    tag=f"th_{side}", bufs=4)
                    nc.scalar.activation(th[:, 0:ng], ps[:, 0:ng], Act.Tanh)
                    ths.append(th)
                    if oc == 0 and prev_score7 is not None:
                        prev_score7()
                    if oc == 1 and prev_tail is not None:
                        prev_tail()
                    if oc > 0:
                        score_mm(oc - 1)
                return (lambda: score_mm(OC - 1)), (lambda: emit_tail(gi, srow))

            def stage2(side, attc, rows0, nr, a_b, off):
                scr2 = wpool.tile([128, n_rows[side]], f16,
                                  name=f"sc2_{side}_{rows0}",
                                  tag=f"scr2_{side}", bufs=2)
                for hc in range(HC):
                    nc.vector.scalar_tensor_tensor(
                        out=scr2[:, 0:nr],
                        in0=xt3[side][:, hc, rows0:rows0 + nr],
                        scalar=1.0, in1=a_b[:, off:off + nr],
                        op0=Alu.mult, op1=Alu.mult,
                        accum_out=attc[:, hc:hc + 1])

            def finalize(side, b, attc, ssum):
                rr = spool.tile([1, 1], f32, name=f"rr_{side}_{b}", tag="rr",
                                bufs=4)
                nc.vector.reciprocal(rr[0:1, 0:1], ssum[0:1, 0:1])
                rb = spool.tile([128, 1], f32, name=f"rb_{side}_{b}", tag="rb",
                                bufs=4)
                nc.gpsimd.partition_broadcast(rb[:, 0:1], rr[0:1, 0:1])
                att = spool.tile([128, HC], f32, name=f"att_{side}_{b}",
                                 tag="att", bufs=2)
                nc.scalar.activation(att[:, 0:HC], attc[:, 0:HC],
                                     Act.Copy, scale=rb[:, 0:1])
                # element h of batch b sits at [h % 128, h // 128]
                nc.sync.dma_start(
                    out=out_d[side][b].rearrange("(hc p) -> p hc", p=128),
                    in_=att[:, 0:HC])

            def emit_tail(gi, srow):
                side, g0, g1, segs = groups[gi]
                ng = g1 - g0
                for b, off, nr, mode in segs:
                    if mode == "a":
                        b3["arow"] = cpool.tile([1, S], f16, name="arow_b3")
                        b3["ab"] = cpool.tile([128, S], f16, name="ab_b3")
                        b3["ssum"] = [spool.tile([1, 1], f32, name=f"ssb3_{h}",
                                                 tag="ssum", bufs=4)
                                      for h in range(2)]
                        b3["attc"] = [spool.tile([128, HC], f32, name=f"atb3_{h}",
                                                 tag="attc", bufs=2)
                                      for h in range(2)]
                    if mode in ("a", "b"):
                        half = 0 if mode == "a" else 1
                        arow, a_b = b3["arow"], b3["ab"]
                        ssum, attc = b3["ssum"][half], b3["attc"][half]
                        ro = half * HS
                    else:
                        arow = spool.tile([1, ng], f16, name=f"arow_{gi}_{b}",
                                          tag=f"arow_{side}", bufs=2)
                        a_b = wpool.tile([128, ng], f16, name=f"ab_{gi}_{b}",
                                         tag=f"ab_{side}", bufs=2)
                        ssum = spool.tile([1, 1], f32, name=f"ss_{gi}_{b}",
                                          tag="ssum", bufs=4)
                        attc = spool.tile([128, HC], f32, name=f"attc_{gi}_{b}",
                                          tag="attc", bufs=2)
                        ro = off
                    nc.scalar.activation(arow[0:1, ro:ro + nr],
                                         srow[0:1, off:off + nr], Act.Exp,
                                         accum_out=ssum[0:1, 0:1])
                    nc.gpsimd.partition_broadcast(a_b[:, ro:ro + nr],
                                                  arow[0:1, ro:ro + nr])
                    stage2(side, attc, g0 + off, nr, a_b, ro)
                    if mode == "full":
                        finalize(side, b, attc, ssum)
                    elif mode == "b":
                        nc.vector.scalar_tensor_tensor(
                            out=b3["ssum"][0][0:1, 0:1], in0=b3["ssum"][0][0:1, 0:1],
                            scalar=1.0, in1=b3["ssum"][1][0:1, 0:1],
                            op0=Alu.mult, op1=Alu.add)
                        nc.vector.scalar_tensor_tensor(
                            out=b3["attc"][0][:, 0:HC], in0=b3["attc"][0][:, 0:HC],
                            scalar=1.0, in1=b3["attc"][1][:, 0:HC],
                            op0=Alu.mult, op1=Alu.add)
                        finalize(side, b, b3["attc"][0], b3["ssum"][0])

            score7, tail = None, None
            for gi in range(len(groups)):
                score7, tail = emit_group(gi, score7, tail)
            score7()
            tail()
    nc.compile()
    return nc


def _get_nc():
    if "nc" not in _CACHE:
        _CACHE["nc"] = build_nc()
    return _CACHE["nc"]


def make_in_maps(inputs):
    dns = np.ascontiguousarray(np.asarray(inputs["dns_feature"], dtype=np.float32))
    img = np.ascontiguousarray(np.asarray(inputs["img_features"], dtype=np.float32))
    W_i1 = np.asarray(inputs["W_i1"], dtype=np.float32)
    W_d2 = np.asarray(inputs["W_d2"], dtype=np.float32)
    wB = np.asarray(inputs["w_att1"], dtype=np.float32)[H:]
    wD = np.asarray(inputs["w_att2"], dtype=np.float32)[H:]

    # W.T [h, o] -> [oc, hc, 128(h), 128(o)]
    def wt_pack(W):
        wt = W.T.reshape(HC, 128, OC, 128).transpose(2, 0, 1, 3)
        return np.ascontiguousarray(wt).astype(_BF16)

    wt_i1 = wt_pack(W_i1)
    wt_d2 = wt_pack(W_d2)
    wcol_b = np.ascontiguousarray(wB.reshape(OC, 128).T).astype(_BF16)
    wcol_d = np.ascontiguousarray(wD.reshape(OC, 128).T).astype(_BF16)

    in_maps = []
    for k in range(NCORES):
        sl = slice(k * BLOC, (k + 1) * BLOC)
        # [BLOC, rows, H] -> [H, BLOC*rows] -> [HC, 128, n]
        xt_d = dns[sl].reshape(BLOC * S, H).T.reshape(HC, 128, BLOC * S)
        xt_i = img[sl].reshape(BLOC * R, H).T.reshape(HC, 128, BLOC * R)
        in_maps.append({
            "xt_dns": np.ascontiguousarray(xt_d).astype(_BF16),
            "xt_img": np.ascontiguousarray(xt_i).astype(_BF16),
            "wt_i1": wt_i1,
            "wt_d2": wt_d2,
            "wcol_b": wcol_b,
            "wcol_d": wcol_d,
        })
    return in_maps


def kernel(**inputs):
    from concourse.bass_utils import run_bass_kernel_spmd

    nc = _get_nc()
    in_maps = make_in_maps(inputs)
    res = run_bass_kernel_spmd(nc, in_maps, list(range(NCORES))).results
    outs = {}
    for name in ("out_dns", "out_img"):
        outs[name] = np.concatenate([res[k][name] for k in range(NCORES)], axis=0)
    out_dns = np.ascontiguousarray(
        np.broadcast_to(outs["out_dns"][:, None, :], (B, S, H)))
    out_img = np.ascontiguousarray(
        np.broadcast_to(outs["out_img"][:, None, :], (B, S, H)))
    return out_dns, out_img


# revision 19
# speedup vs baseline: 1.1029x; 1.1029x over previous
"""CoAttention ImageDNS kernel for Trainium2 (8 NeuronCores, Bass/Tile).

Math: the reference computes two additive-attention blocks. In both, the
softmax'd score is  score[b, q, k] = f(q-side)[b, q] + g(k-side)[b, k] + c,
and softmax over k is invariant to the q-dependent (and constant) terms, so
the attention weights are independent of the query index:

  visual_att[b, s, :]  = softmax_r( wB . tanh(W_i1 @ img[b, r]) )
  textual_att[b, i, :] = softmax_j( wD . tanh(W_d2 @ dns[b, j]) )

Hence both outputs are per-batch rank-1 broadcasts:

  att_img_features[b, s, :] = visual_att[b]  @ img[b]   (same for all s)
  att_dns_features[b, i, :] = textual_att[b] @ dns[b]   (same for all i)

W_d1/b_d1/w_att1[:H]/b_att1/W_i2/b_i2/w_att2[:H]/b_att2 cancel entirely.

Sharding: pure data-parallel over batch, 4 batches per core, no collectives.
The device computes the per-batch [H] attention outputs; the host broadcasts
them over the (identical) S query rows, so the kernel writes only B*H values
instead of B*S*H.

Layout: projections run transposed (weights stationary, activations moving):
proj^T[o, r] accumulates per 128-wide o-chunk over the h-chunks, so the
score reduction over o is 8 tiny PE matmuls against the w-column and the
scores land in ROW layout on partition 0.  From there: exp (+softmax sum via
accum) on Scalar, partition-broadcast of the weight row on GpSimd, stage-2
weighted row sums on Vector (stt accum over the same transposed activation
tiles the projections use - the natural-layout activations are never
loaded).  The PE stream is pure projection + score matmuls; each group's
softmax/stage-2 tail is emitted one group later so it hides under the next
group's projections.  The img side (2-batch groups) runs first so the first
matmul only waits on ~1MB of DMA; the last dns batch is split into two
256-row sub-groups so the final exposed tail is one half-batch of vector
work.  HBM in is ~10MB/core.
"""

import sys
import numpy as np
import ml_dtypes

_BF16 = ml_dtypes.bfloat16

for _p in ("/opt/trn_rl_repo", "/root/.axon_site/_ro/trn_rl_repo"):
    if _p not in sys.path:
        sys.path.append(_p)

B, S, R, H = 32, 512, 196, 1024
NCORES = 8
BLOC = B // NCORES          # batches per core
HC = H // 128               # contraction chunks of 128
OC = H // 128               # projection output chunks of 128
NI = BLOC * R               # img rows, all batches packed (784)
ND = BLOC * S               # dns rows, all batches packed (2048)
GI = 2 * R                  # img group rows (2 batches)
HS = S // 2                 # dns sub-group rows for the final batch (256)

_CACHE = {}


def build_nc():
    from concourse import bacc, mybir
    from concourse import tile

    f32, f16 = mybir.dt.float32, mybir.dt.bfloat16
    Act = mybir.ActivationFunctionType
    Alu = mybir.AluOpType

    nc = bacc.Bacc("TRN2", target_bir_lowering=False, debug=False)

    xt_img = nc.dram_tensor("xt_img", [HC, 128, NI], f16, kind="ExternalInput")
    xt_dns = nc.dram_tensor("xt_dns", [HC, 128, ND], f16, kind="ExternalInput")
    # oc-major weight layout: [oc, hc, 128(h), 128(o)] so one 0.25MB DMA
    # delivers everything one proj accumulation group needs
    wt_i1 = nc.dram_tensor("wt_i1", [OC, HC, 128, 128], f16, kind="ExternalInput")
    wt_d2 = nc.dram_tensor("wt_d2", [OC, HC, 128, 128], f16, kind="ExternalInput")
    wcol_b = nc.dram_tensor("wcol_b", [128, OC], f16, kind="ExternalInput")
    wcol_d = nc.dram_tensor("wcol_d", [128, OC], f16, kind="ExternalInput")
    # [p, b, hc] layout: element h of batch b lives at [h % 128, b, h // 128];
    # contiguous 32B runs per partition (the [b, h] layout would be a 4-byte
    # scatter with 512B stride - ~9us of DMA RMW per batch on HW)
    out_dns = nc.dram_tensor("out_dns", [128, BLOC, HC], f32, kind="ExternalOutput")
    out_img = nc.dram_tensor("out_img", [128, BLOC, HC], f32, kind="ExternalOutput")

    with tile.TileContext(nc) as tc:
        with (
            tc.tile_pool(name="const", bufs=1) as cpool,
            tc.tile_pool(name="work", bufs=3) as wpool,
            tc.tile_pool(name="small", bufs=8) as spool,
            tc.tile_pool(name="ppd", bufs=3, space="PSUM") as ppd,
            tc.tile_pool(name="ppi", bufs=3, space="PSUM") as ppi,
            tc.tile_pool(name="psr", bufs=2, space="PSUM") as psr,
        ):
            xt_i = cpool.tile([128, HC * NI], f16, name="xt_img_sb")
            xt_d = cpool.tile([128, HC * ND], f16, name="xt_dns_sb")
            wt_sb = {"img": cpool.tile([128, OC * HC * 128], f16, name="wt_i1_sb"),
                     "dns": cpool.tile([128, OC * HC * 128], f16, name="wt_d2_sb")}
            wc_sb = {"img": cpool.tile([128, OC], f16, name="wcol_b_sb"),
                     "dns": cpool.tile([128, OC], f16, name="wcol_d_sb")}
            att_sb = {s: cpool.tile([128, BLOC * HC], f32, name=f"att_{s}_sb")
                      for s in ("img", "dns")}

            wt_dram = {"img": wt_i1, "dns": wt_d2}
            wc_dram = {"img": wcol_b, "dns": wcol_d}
            xt_dram = {"img": xt_img, "dns": xt_dns}
            out_d = {"img": out_img, "dns": out_dns}
            n_rows = {"img": R, "dns": S}

            def load_wt_oc(side, oc, eng=None):
                w = wt_sb[side]
                (eng or nc.sync).dma_start(
                    out=w[:, oc * HC * 128:(oc + 1) * HC * 128]
                    .rearrange("p (hc m) -> p hc m", hc=HC),
                    in_=wt_dram[side][oc].rearrange("hc p m -> p hc m"))

            def load_xt(side, c0, c1, h0=0, h1=HC):
                nc.sync.dma_start(
                    out=xt3[side][:, h0:h1, c0:c1],
                    in_=xt_dram[side][h0:h1, :, c0:c1]
                    .rearrange("hc p m -> p hc m"))

            wt3 = {s: wt_sb[s].rearrange("p (oc hc m) -> p oc hc m", oc=OC, hc=HC)
                   for s in ("img", "dns")}
            xt3 = {"img": xt_i.rearrange("p (hc m) -> p hc m", hc=HC),
                   "dns": xt_d.rearrange("p (hc m) -> p hc m", hc=HC)}

            # groups: (side, row0, row1, tail segments (batch, off, nr, mode))
            # mode: "full" = standalone batch, "a"/"b" = halves of dns b3
            groups = [
                ("img", 0 * GI, 1 * GI, [(0, 0, R, "full"), (1, R, R, "full")]),
                ("img", 1 * GI, 2 * GI, [(2, 0, R, "full"), (3, R, R, "full")]),
                ("dns", 0 * S, 1 * S, [(0, 0, S, "full")]),
                ("dns", 1 * S, 2 * S, [(1, 0, S, "full")]),
                ("dns", 2 * S, 3 * S, [(2, 0, S, "full")]),
                ("dns", 3 * S, 3 * S + HS, [(3, 0, HS, "a")]),
                ("dns", 3 * S + HS, 4 * S, [(3, 0, HS, "b")]),
            ]

            def emit_loads(gi):
                side = groups[gi][0]
                if gi == 0:
                    # first weight chunks ride the scalar engine's DGE queue
                    # so they land in parallel with the first xt loads
                    for oc in range(0, 4):
                        load_wt_oc("img", oc, eng=nc.scalar)
                    # halves so the first proj matmuls start sooner
                    load_xt("img", 0, GI, 0, HC // 2)
                    load_xt("img", 0, GI, HC // 2, HC)
                    nc.sync.dma_start(out=wc_sb["img"][:, :],
                                      in_=wc_dram["img"][:, :])
                    for oc in range(4, OC):
                        load_wt_oc("img", oc)
                elif gi == 1:
                    load_xt("img", GI, NI)
                    nc.sync.dma_start(out=wc_sb["dns"][:, :],
                                      in_=wc_dram["dns"][:, :])
                elif gi == 2:
                    load_xt("dns", 0, S)
                    for oc in range(OC):
                        load_wt_oc("dns", oc)
                elif gi in (3, 4):
                    g0, g1 = groups[gi][1], groups[gi][2]
                    load_xt("dns", g0, g1)
                elif gi == 5:
                    load_xt("dns", 3 * S, 4 * S)

            # dedicated state for the split final dns batch
            b3 = {}

            def emit_group(gi, prev_score7, prev_tail):
                side, g0, g1, _ = groups[gi]
                ng = g1 - g0
                pp = ppd if side == "dns" else ppi
                npp = S if side == "dns" else GI
                srow = psr.tile([1, S], f32, name=f"srow_{gi}", tag="srow")
                ths = []

                def score_mm(oc):
                    nc.tensor.matmul(
                        srow[0:1, 0:ng], lhsT=wc_sb[side][:, oc:oc + 1],
                        rhs=ths[oc][:, 0:ng],
                        start=(oc == 0), stop=(oc == OC - 1))

                emit_loads(gi)
                for oc in range(OC):
                    ps = pp.tile([128, npp], f32, name=f"proj_{gi}_{oc}",
                                 tag=f"pp_{side}")
                    for hc in range(HC):
                        nc.tensor.matmul(
                            ps[:, 0:ng],
                            lhsT=wt3[side][:, oc, hc, :],
                            rhs=xt3[side][:, hc, g0:g1],
                            start=(hc == 0), stop=(hc == HC - 1))
                    th = wpool.tile([128, npp], f16, name=f"th_{gi}_{oc}",
                                    tag=f"th_{side}", bufs=4)
                    nc.scalar.activation(th[:, 0:ng], ps[:, 0:ng], Act.Tanh)
                    ths.append(th)
                    if oc == 0 and prev_score7 is not None:
                        prev_score7()
                    if oc == 1 and prev_tail is not None:
                        prev_tail()
                    if oc > 0:
                        score_mm(oc - 1)
                return (lambda: score_mm(OC - 1)), (lambda: emit_tail(gi, srow))

            def stage2(side, attc, rows0, nr, a_b, off):
                scr2 = wpool.tile([128, n_rows[side]], f16,
                                  name=f"sc2_{side}_{rows0}",
                                  tag=f"scr2_{side}", bufs=2)
                for hc in range(HC):
                    nc.vector.scalar_tensor_tensor(
                        out=scr2[:, 0:nr],
                        in0=xt3[side][:, hc, rows0:rows0 + nr],
                        scalar=1.0, in1=a_b[:, off:off + nr],
                        op0=Alu.mult, op1=Alu.mult,
                        accum_out=attc[:, hc:hc + 1])

            def finalize(side, b, attc, ssum):
                rr = spool.tile([1, 1], f32, name=f"rr_{side}_{b}", tag="rr",
                                bufs=4)
                nc.vector.reciprocal(rr[0:1, 0:1], ssum[0:1, 0:1])
                rb = spool.tile([128, 1], f32, name=f"rb_{side}_{b}", tag="rb",
                                bufs=4)
                nc.gpsimd.partition_broadcast(rb[:, 0:1], rr[0:1, 0:1])
                nc.scalar.activation(
                    att_sb[side][:, b * HC:(b + 1) * HC], attc[:, 0:HC],
                    Act.Copy, scale=rb[:, 0:1])

            def emit_tail(gi, srow):
                side, g0, g1, segs = groups[gi]
                ng = g1 - g0
                for b, off, nr, mode in segs:
                    if mode == "a":
                        b3["arow"] = cpool.tile([1, S], f16, name="arow_b3")
                        b3["ab"] = cpool.tile([128, S], f16, name="ab_b3")
                        b3["ssum"] = [spool.tile([1, 1], f32, name=f"ssb3_{h}",
                                                 tag="ssum", bufs=4)
                                      for h in range(2)]
                        b3["attc"] = [spool.tile([128, HC], f32, name=f"atb3_{h}",
                                                 tag="attc", bufs=2)
                                      for h in range(2)]
                    if mode in ("a", "b"):
                        half = 0 if mode == "a" else 1
                        arow, a_b = b3["arow"], b3["ab"]
                        ssum, attc = b3["ssum"][half], b3["attc"][half]
                        ro = half * HS
                    else:
                        arow = spool.tile([1, ng], f16, name=f"arow_{gi}_{b}",
                                          tag=f"arow_{side}", bufs=2)
                        a_b = wpool.tile([128, ng], f16, name=f"ab_{gi}_{b}",
                                         tag=f"ab_{side}", bufs=2)
                        ssum = spool.tile([1, 1], f32, name=f"ss_{gi}_{b}",
                                          tag="ssum", bufs=4)
                        attc = spool.tile([128, HC], f32, name=f"attc_{gi}_{b}",
                                          tag="attc", bufs=2)
                        ro = off
                    nc.scalar.activation(arow[0:1, ro:ro + nr],
                                         srow[0:1, off:off + nr], Act.Exp,
                                         accum_out=ssum[0:1, 0:1])
                    nc.gpsimd.partition_broadcast(a_b[:, ro:ro + nr],
                                                  arow[0:1, ro:ro + nr])
                    stage2(side, attc, g0 + off, nr, a_b, ro)
                    if mode == "full":
                        finalize(side, b, attc, ssum)
                    elif mode == "b":
                        nc.vector.scalar_tensor_tensor(
                            out=b3["ssum"][0][0:1, 0:1], in0=b3["ssum"][0][0:1, 0:1],
                            scalar=1.0, in1=b3["ssum"][1][0:1, 0:1],
                            op0=Alu.mult, op1=Alu.add)
                        nc.vector.scalar_tensor_tensor(
                            out=b3["attc"][0][:, 0:HC], in0=b3["attc"][0][:, 0:HC],
                            scalar=1.0, in1=b3["attc"][1][:, 0:HC],
                            op0=Alu.mult, op1=Alu.add)
                        finalize(side, b, b3["attc"][0], b3["ssum"][0])

            score7, tail = None, None
            for gi in range(len(groups)):
                score7, tail = emit_group(gi, score7, tail)
            score7()
            tail()

            for side in ("img", "dns"):
                nc.sync.dma_start(
                    out=out_d[side].rearrange("p b hc -> p (b hc)"),
                    in_=att_sb[side][:, :])
    nc.compile()
    return nc


def _get_nc():
    if "nc" not in _CACHE:
        _CACHE["nc"] = build_nc()
    return _CACHE["nc"]


def make_in_maps(inputs):
    dns = np.ascontiguousarray(np.asarray(inputs["dns_feature"], dtype=np.float32))
    img = np.ascontiguousarray(np.asarray(inputs["img_features"], dtype=np.float32))
    W_i1 = np.asarray(inputs["W_i1"], dtype=np.float32)
    W_d2 = np.asarray(inputs["W_d2"], dtype=np.float32)
    wB = np.asarray(inputs["w_att1"], dtype=np.float32)[H:]
    wD = np.asarray(inputs["w_att2"], dtype=np.float32)[H:]

    # W.T [h, o] -> [oc, hc, 128(h), 128(o)]
    def wt_pack(W):
        wt = W.T.reshape(HC, 128, OC, 128).transpose(2, 0, 1, 3)
        return np.ascontiguousarray(wt).astype(_BF16)

    wt_i1 = wt_pack(W_i1)
    wt_d2 = wt_pack(W_d2)
    wcol_b = np.ascontiguousarray(wB.reshape(OC, 128).T).astype(_BF16)
    wcol_d = np.ascontiguousarray(wD.reshape(OC, 128).T).astype(_BF16)

    in_maps = []
    for k in range(NCORES):
        sl = slice(k * BLOC, (k + 1) * BLOC)
        # [BLOC, rows, H] -> [H, BLOC*rows] -> [HC, 128, n]
        xt_d = dns[sl].reshape(BLOC * S, H).T.reshape(HC, 128, BLOC * S)
        xt_i = img[sl].reshape(BLOC * R, H).T.reshape(HC, 128, BLOC * R)
        in_maps.append({
            "xt_dns": np.ascontiguousarray(xt_d).astype(_BF16),
            "xt_img": np.ascontiguousarray(xt_i).astype(_BF16),
            "wt_i1": wt_i1,
            "wt_d2": wt_d2,
            "wcol_b": wcol_b,
            "wcol_d": wcol_d,
        })
    return in_maps


def kernel(**inputs):
    from concourse.bass_utils import run_bass_kernel_spmd

    nc = _get_nc()
    in_maps = make_in_maps(inputs)
    res = run_bass_kernel_spmd(nc, in_maps, list(range(NCORES))).results
    # device out: [128, BLOC, HC], element h of batch b at [h % 128, b, h//128]
    outs = {}
    for name in ("out_dns", "out_img"):
        per = [res[k][name].transpose(1, 2, 0).reshape(BLOC, H)
               for k in range(NCORES)]
        outs[name] = np.concatenate(per, axis=0)
    out_dns = np.ascontiguousarray(
        np.broadcast_to(outs["out_dns"][:, None, :], (B, S, H)))
    out_img = np.ascontiguousarray(
        np.broadcast_to(outs["out_img"][:, None, :], (B, S, H)))
    return out_dns, out_img


# revision 27
# speedup vs baseline: 1.2164x; 1.1030x over previous
"""CoAttention ImageDNS kernel for Trainium2 (8 NeuronCores, Bass/Tile).

Math: the reference computes two additive-attention blocks. In both, the
softmax'd score is  score[b, q, k] = f(q-side)[b, q] + g(k-side)[b, k] + c,
and softmax over k is invariant to the q-dependent (and constant) terms, so
the attention weights are independent of the query index:

  visual_att[b, s, :]  = softmax_r( wB . tanh(W_i1 @ img[b, r]) )
  textual_att[b, i, :] = softmax_j( wD . tanh(W_d2 @ dns[b, j]) )

Hence both outputs are per-batch rank-1 broadcasts:

  att_img_features[b, s, :] = visual_att[b]  @ img[b]   (same for all s)
  att_dns_features[b, i, :] = textual_att[b] @ dns[b]   (same for all i)

W_d1/b_d1/w_att1[:H]/b_att1/W_i2/b_i2/w_att2[:H]/b_att2 cancel entirely.

Sharding: pure data-parallel over batch, 4 batches per core, no collectives.
The device computes the per-batch [H] attention outputs; the host broadcasts
them over the (identical) S query rows, so the kernel writes only B*H values
instead of B*S*H.

Layout: projections keep activations stationary (lhsT = x^T chunk, reused
across both 512-wide output halves, so LDWEIGHTS stays hidden) and stream
the weights; proj rows land on PSUM partitions.  Scores: tanh (Scalar, bf16)
then a weighted free-dim reduction (Vector stt accum); exp'd score columns
are broadcast to [128, rows] with one tiny PE matmul per chunk
(a-column x identity), softmax sums come free from a Scalar Copy+accum over
the broadcast tile, and stage-2 weighted row sums run on Vector over the
same transposed activation tiles the projections use (the natural-layout
activations are never loaded).  Each group's tail is emitted one group later
so it hides under the next group's projections; rows are packed across
batches (img: all 4 batches = 7 row chunks) to minimize M-padding.
HBM in is ~10MB/core.
"""

import sys
import numpy as np
import ml_dtypes

_BF16 = ml_dtypes.bfloat16

for _p in ("/opt/trn_rl_repo", "/root/.axon_site/_ro/trn_rl_repo"):
    if _p not in sys.path:
        sys.path.append(_p)

B, S, R, H = 32, 512, 196, 1024
NCORES = 8
BLOC = B // NCORES          # batches per core
HC = H // 128               # contraction chunks of 128
OC = 512                    # output-chunk (one fp32 PSUM bank)
NI = BLOC * R               # img rows, all batches packed (784)
ND = BLOC * S               # dns rows, all batches packed (2048)

_CACHE = {}


def _row_chunks(n):
    out, o = [], 0
    while o < n:
        out.append((o, min(128, n - o)))
        o += 128
    return out


def build_nc():
    from concourse import bacc, mybir
    from concourse import tile

    f32, f16 = mybir.dt.float32, mybir.dt.bfloat16
    Act = mybir.ActivationFunctionType
    Alu = mybir.AluOpType

    nc = bacc.Bacc("TRN2", target_bir_lowering=False, debug=False)

    xt_img = nc.dram_tensor("xt_img", [HC, 128, NI], f16, kind="ExternalInput")
    xt_dns = nc.dram_tensor("xt_dns", [HC, 128, ND], f16, kind="ExternalInput")
    wt_i1 = nc.dram_tensor("wt_i1", [HC, 128, H], f16, kind="ExternalInput")
    wt_d2 = nc.dram_tensor("wt_d2", [HC, 128, H], f16, kind="ExternalInput")
    wrow_b = nc.dram_tensor("wrow_b", [128, H], f16, kind="ExternalInput")
    wrow_d = nc.dram_tensor("wrow_d", [128, H], f16, kind="ExternalInput")
    ident_d = nc.dram_tensor("ident", [128, 128], f16, kind="ExternalInput")
    # [p, b, hc] layout: element h of batch b lives at [h % 128, b, h // 128];
    # contiguous 32B runs per partition (a [b, h] layout would be a 4-byte
    # scatter with 512B stride - ~9us of DMA RMW per batch on HW)
    out_dns = nc.dram_tensor("out_dns", [128, BLOC, HC], f32, kind="ExternalOutput")
    out_img = nc.dram_tensor("out_img", [128, BLOC, HC], f32, kind="ExternalOutput")

    with tile.TileContext(nc) as tc:
        with (
            tc.tile_pool(name="const", bufs=1) as cpool,
            tc.tile_pool(name="work", bufs=3) as wpool,
            tc.tile_pool(name="small", bufs=8) as spool,
            tc.tile_pool(name="pp", bufs=2, space="PSUM") as ppool,
            tc.tile_pool(name="pt", bufs=1, space="PSUM") as ptps,
        ):
            xt_i = cpool.tile([128, HC * NI], f16, name="xt_img_sb")
            xt_d = cpool.tile([128, HC * ND], f16, name="xt_dns_sb")
            wt_sb = {"img": cpool.tile([128, HC * H], f16, name="wt_i1_sb"),
                     "dns": cpool.tile([128, HC * H], f16, name="wt_d2_sb")}
            wr_sb = {"img": cpool.tile([128, H], f16, name="wrow_b_sb"),
                     "dns": cpool.tile([128, H], f16, name="wrow_d_sb")}
            ident = cpool.tile([128, 128], f16, name="ident_sb")
            att_sb = {s: cpool.tile([128, BLOC * HC], f32, name=f"att_{s}_sb")
                      for s in ("img", "dns")}

            wt_dram = {"img": wt_i1, "dns": wt_d2}
            wr_dram = {"img": wrow_b, "dns": wrow_d}
            xt_dram = {"img": xt_img, "dns": xt_dns}
            out_d = {"img": out_img, "dns": out_dns}
            n_rows = {"img": R, "dns": S}

            def load_wt_hc(side, hc, eng=None):
                (eng or nc.sync).dma_start(
                    out=wt_sb[side][:, hc * H:(hc + 1) * H],
                    in_=wt_dram[side][hc])

            def load_xt(side, c0, c1, h0=0, h1=HC, eng=None):
                (eng or nc.sync).dma_start(
                    out=xt3[side][:, h0:h1, c0:c1],
                    in_=xt_dram[side][h0:h1, :, c0:c1]
                    .rearrange("hc p m -> p hc m"))

            wt3 = {s: wt_sb[s].rearrange("p (hc m) -> p hc m", hc=HC)
                   for s in ("img", "dns")}
            xt3 = {"img": xt_i.rearrange("p (hc m) -> p hc m", hc=HC),
                   "dns": xt_d.rearrange("p (hc m) -> p hc m", hc=HC)}

            # groups: (side, row0, row1, tail segments (batch, off, nr));
            # img packs all 4 batches (7 row chunks instead of 8)
            groups = [
                ("img", 0, NI, [(b, b * R, R) for b in range(BLOC)]),
                ("dns", 0 * S, 1 * S, [(0, 0, S)]),
                ("dns", 1 * S, 2 * S, [(1, 0, S)]),
                ("dns", 2 * S, 3 * S, [(2, 0, S)]),
                ("dns", 3 * S, 4 * S, [(3, 0, S)]),
            ]

            def emit_loads(gi):
                side = groups[gi][0]
                if gi == 0:
                    # ramp: interleave per-hc weight/activation chunks over
                    # two DGE queues so the first proj group starts early
                    for hc in range(HC):
                        load_wt_hc("img", hc,
                                   eng=nc.scalar if hc % 2 else nc.sync)
                        load_xt("img", 0, NI, hc, hc + 1,
                                eng=nc.sync if hc % 2 else nc.scalar)
                    nc.sync.dma_start(out=ident[:, :], in_=ident_d[:, :])
                    nc.sync.dma_start(out=wr_sb["img"][:, :],
                                      in_=wr_dram["img"][:, :])
                elif gi == 1:
                    load_xt("dns", 0, S)
                    for hc in range(HC):
                        load_wt_hc("dns", hc)
                    nc.sync.dma_start(out=wr_sb["dns"][:, :],
                                      in_=wr_dram["dns"][:, :])
                else:
                    g0, g1 = groups[gi][1], groups[gi][2]
                    load_xt("dns", g0, g1)

            def emit_group(gi, prev_tail):
                side, g0, g1, _ = groups[gi]
                rcs = _row_chunks(g1 - g0)
                acs = spool.tile([128, len(rcs)], f16, name=f"acs_{gi}",
                                 tag=f"acs_{side}", bufs=2)
                emit_loads(gi)
                for ci, (c0, rk) in enumerate(rcs):
                    r0 = g0 + c0
                    ps = ppool.tile([128, H], f32, name=f"proj_{gi}_{ci}",
                                    tag="pp")
                    for hc in range(HC):
                        lhs = xt3[side][:, hc, r0:r0 + rk]
                        for o2 in range(2):
                            nc.tensor.matmul(
                                ps[0:rk, o2 * OC:(o2 + 1) * OC],
                                lhsT=lhs,
                                rhs=wt3[side][:, hc, o2 * OC:(o2 + 1) * OC],
                                start=(hc == 0), stop=(hc == HC - 1))
                    th = wpool.tile([128, H], f16, name=f"th_{gi}_{ci}",
                                    tag="th", bufs=3)
                    nc.scalar.activation(th[0:rk, :], ps[0:rk, :], Act.Tanh)
                    scr = wpool.tile([128, H], f16, name=f"scr_{gi}_{ci}",
                                     tag="scr", bufs=2)
                    tcol = spool.tile([128, 1], f32, name=f"tc_{gi}_{ci}",
                                      tag="tcol", bufs=4)
                    nc.vector.scalar_tensor_tensor(
                        out=scr[0:rk, :], in0=th[0:rk, :], scalar=1.0,
                        in1=wr_sb[side][0:rk, :], op0=Alu.mult, op1=Alu.mult,
                        accum_out=tcol[0:rk, :])
                    nc.scalar.activation(acs[0:rk, ci:ci + 1], tcol[0:rk, :],
                                         Act.Exp)
                    if ci == 1 and prev_tail is not None:
                        prev_tail()
                return lambda: emit_tail(gi, acs, rcs)

            def emit_tail(gi, acs, rcs):
                side, g0, g1, segs = groups[gi]
                ng = g1 - g0
                # broadcast each exp'd score column to [128, rk] rows via a
                # tiny PE matmul (a-col x identity)
                ab_ps = ptps.tile([128, ng], f32, name=f"abps_{gi}",
                                  tag=f"abps_{side}",
                                  bufs=1 if side == "img" else 2)
                for ci, (c0, rk) in enumerate(rcs):
                    nc.tensor.matmul(
                        ab_ps[:, c0:c0 + rk],
                        lhsT=acs[0:rk, ci:ci + 1].to_broadcast((rk, 128)),
                        rhs=ident[0:rk, 0:rk], start=True, stop=True)
                a_b = wpool.tile([128, ng], f16, name=f"ab_{gi}",
                                 tag=f"ab_{side}", bufs=1 if side == "img" else 2)
                nc.vector.tensor_copy(a_b[:, :], ab_ps[:, :])
                for b, off, nr in segs:
                    # softmax sum: every partition of a_b holds the full
                    # weight row, so a Copy+accum gives the sum broadcast
                    scrap = wpool.tile([128, n_rows[side]], f16,
                                       name=f"scrap_{gi}_{b}", tag="scrap",
                                       bufs=2)
                    asum = spool.tile([128, 1], f32, name=f"as_{gi}_{b}",
                                      tag="asum", bufs=4)
                    nc.scalar.activation(scrap[:, 0:nr], a_b[:, off:off + nr],
                                         Act.Copy, accum_out=asum[:, 0:1])
                    rb = spool.tile([128, 1], f32, name=f"rb_{gi}_{b}",
                                    tag="rb", bufs=4)
                    nc.vector.reciprocal(rb[:, 0:1], asum[:, 0:1])
                    attc = spool.tile([128, HC], f32, name=f"attc_{gi}_{b}",
                                      tag="attc", bufs=2)
                    scr2 = wpool.tile([128, n_rows[side]], f16,
                                      name=f"sc2_{gi}_{b}", tag="scr2", bufs=2)
                    for hc in range(HC):
                        nc.vector.scalar_tensor_tensor(
                            out=scr2[:, 0:nr],
                            in0=xt3[side][:, hc, g0 + off:g0 + off + nr],
                            scalar=1.0, in1=a_b[:, off:off + nr],
                            op0=Alu.mult, op1=Alu.mult,
                            accum_out=attc[:, hc:hc + 1])
                    nc.scalar.activation(
                        att_sb[side][:, b * HC:(b + 1) * HC], attc[:, 0:HC],
                        Act.Copy, scale=rb[:, 0:1])
                if side == "img" or segs[0][0] == BLOC - 1:
                    nc.sync.dma_start(
                        out=out_d[side].rearrange("p b hc -> p (b hc)"),
                        in_=att_sb[side][:, :])

            tail = None
            for gi in range(len(groups)):
                tail = emit_group(gi, tail)
            tail()
    nc.compile()
    return nc


def _get_nc():
    if "nc" not in _CACHE:
        _CACHE["nc"] = build_nc()
    return _CACHE["nc"]


def make_in_maps(inputs):
    dns = np.ascontiguousarray(np.asarray(inputs["dns_feature"], dtype=np.float32))
    img = np.ascontiguousarray(np.asarray(inputs["img_features"], dtype=np.float32))
    W_i1 = np.asarray(inputs["W_i1"], dtype=np.float32)
    W_d2 = np.asarray(inputs["W_d2"], dtype=np.float32)
    wB = np.asarray(inputs["w_att1"], dtype=np.float32)[H:]
    wD = np.asarray(inputs["w_att2"], dtype=np.float32)[H:]

    wt_i1 = np.ascontiguousarray(W_i1.T).reshape(HC, 128, H).astype(_BF16)
    wt_d2 = np.ascontiguousarray(W_d2.T).reshape(HC, 128, H).astype(_BF16)
    wrow_b = np.ascontiguousarray(np.broadcast_to(wB, (128, H))).astype(_BF16)
    wrow_d = np.ascontiguousarray(np.broadcast_to(wD, (128, H))).astype(_BF16)
    ident = np.eye(128, dtype=_BF16)

    in_maps = []
    for k in range(NCORES):
        sl = slice(k * BLOC, (k + 1) * BLOC)
        # [BLOC, rows, H] -> [H, BLOC*rows] -> [HC, 128, n]
        xt_d = dns[sl].reshape(BLOC * S, H).T.reshape(HC, 128, BLOC * S)
        xt_i = img[sl].reshape(BLOC * R, H).T.reshape(HC, 128, BLOC * R)
        in_maps.append({
            "xt_dns": np.ascontiguousarray(xt_d).astype(_BF16),
            "xt_img": np.ascontiguousarray(xt_i).astype(_BF16),
            "wt_i1": wt_i1,
            "wt_d2": wt_d2,
            "wrow_b": wrow_b,
            "wrow_d": wrow_d,
            "ident": ident,
        })
    return in_maps


def kernel(**inputs):
    from concourse.bass_utils import run_bass_kernel_spmd

    nc = _get_nc()
    in_maps = make_in_maps(inputs)
    res = run_bass_kernel_spmd(nc, in_maps, list(range(NCORES))).results
    # device out: [128, BLOC, HC], element h of batch b at [h % 128, b, h//128]
    outs = {}
    for name in ("out_dns", "out_img"):
        per = [res[k][name].transpose(1, 2, 0).reshape(BLOC, H)
               for k in range(NCORES)]
        outs[name] = np.concatenate(per, axis=0)
    out_dns = np.ascontiguousarray(
        np.broadcast_to(outs["out_dns"][:, None, :], (B, S, H)))
    out_img = np.ascontiguousarray(
        np.broadcast_to(outs["out_img"][:, None, :], (B, S, H)))
    return out_dns, out_img
